# revision 9
# baseline (speedup 1.0000x reference)
"""DelayRNN Trainium2 kernel (v2).

Sharding (hardcoded from spec): data-parallel over batch. B=32 rows are
sharded 4-per-core across 8 NeuronCores; every core holds all weights in
SBUF and runs the full 256-step encode + 64-step decode recurrence for its
4 rows. No cross-core communication.

Math reformulation (validated vs reference):
  Wh = W_in[:H], Wx = W_in[H:]
  Wp2 = Wh @ W_pass ; dW = Wp2 - Wh        (folded, on device)
  cx_t = x_t @ Wx + b_in ;  cp_t = x_t @ (Wx@W_pass) + (b_in@W_pass+b_pass)
  cmix_t = m_t ? cp_t : cx_t               (precomputed batched, in DRAM)
  h'_t = h0@Wh + (m*h0)@dW + cmix_t        (PSUM-accumulated; cmix enters
                                            PSUM via a K=4 identity matmul)
  tau' = max(16*sigmoid(h'@W_tau + b_tau), 1); mem = sigmoid(h'@W_mem+b_mem)
  h0_next = buf[0] + (mem*h')/tau'
  nbuf[0:15] = buf[1:16] + q*r[1:16], q = mem*h', r_d = 1/(1+|tau'-d|)
Decode: h' = h0@Wp2 + cdec; out_t = h0@W_out + b_out batched at the end.

Performance structure (per step):
- L1/L2 matmul streams use 2x column tiling (tile_position col groups) so
  two weight streams flow through the PE concurrently via separate XBUSes.
  L1 splits by N-half: group (0,0) -> psum[0:4, 0:256] accumulates ALL
  half-0 terms (incl. cmix identity-MM), group (0,32) -> psum[32:36,
  256:512]. h' then needs only plain PSUM->SBUF copies (DVE half / ACT
  half, concurrent engines), no partial-sum adds.
- L2 runs tau in col group (0,64) and mem in (0,96) (bf16 weights, N=256
  halves) so the per-half psum copies overlap the other half's stream.
- tau/mem transposes (T2) are bf16; sigmoid reads the T2 PSUM directly.
- q reads the T1 PSUM directly so the h'-recurrence stays fp32r-exact;
  only the sigmoid paths see bf16 rounding.
- The buffer weight chain w = q/(1+|tau-d|) runs on Pool (gpsimd) +
  one DVE reciprocal_approx_fast, overlapped with the next step's L1.
fp32r moving streams need N>=256 for 1 cyc/row; bf16 has no such limit.
"""

import sys
import numpy as np

for _p in ("/opt/trn_rl_repo",):
    if _p not in sys.path:
        sys.path.append(_p)

from contextlib import ExitStack

import concourse.bass as bass
import concourse.tile as tile
from concourse import bacc, mybir
from concourse.masks import make_identity

FP32 = mybir.dt.float32
FP32R = mybir.dt.float32r
BF16 = mybir.dt.bfloat16
I32 = mybir.dt.int32

B, S, I, H, C = 32, 256, 128, 512, 64
T_OUT = 64
NCORES = 8
BL = B // NCORES        # 4 batch rows per core
KC = H // 128           # 4 k-chunks
D = 16                  # delay slots 1..16
CB = KC * BL            # 16 = chunks x batch (free size of ^T tiles)
NH = H // 2             # 256: N-half for L1/L2 streams

Sig = mybir.ActivationFunctionType.Sigmoid
Op = mybir.AluOpType


def f32(ap):
    return ap.bitcast(FP32)


def build(seq_len=S, t_out=T_OUT, zero_bias=True, reps=1):
    nc = bacc.Bacc("TRN2", target_bir_lowering=False, debug=False)

    # ---------------- DRAM I/O ----------------
    dx = nc.dram_tensor("x", [BL, seq_len, I], FP32, kind="ExternalInput")
    dlen = nc.dram_tensor("lengths", [BL], I32, kind="ExternalInput")
    dwin = nc.dram_tensor("W_in", [I + H, H], FP32, kind="ExternalInput")
    dwpass = nc.dram_tensor("W_pass", [H, H], FP32, kind="ExternalInput")
    dwtau = nc.dram_tensor("W_tau", [H, H], FP32, kind="ExternalInput")
    dwmem = nc.dram_tensor("W_mem", [H, H], FP32, kind="ExternalInput")
    dwout = nc.dram_tensor("W_out", [H, C], FP32, kind="ExternalInput")
    dbias = {}
    for nm, ln in [("b_in", H), ("b_pass", H), ("b_tau", H),
                   ("b_mem", H), ("b_out", C)]:
        dbias[nm] = nc.dram_tensor(nm, [ln], FP32, kind="ExternalInput")
    dout = nc.dram_tensor("out", [BL, t_out, C], FP32, kind="ExternalOutput")
    # internal DRAM scratch (fp32r so the per-step reload is pre-rounded)
    dcmix = nc.dram_tensor("cmix_scratch", [BL * seq_len, H], BF16)

    NROW = BL * seq_len            # bt rows
    NMT = NROW // 128              # row tiles for the cx/cp precompute
    TPB = seq_len // 128           # row tiles per batch row

    with tile.TileContext(nc) as tc, ExitStack() as ctx:
        persist = ctx.enter_context(tc.tile_pool(name="persist", bufs=1))

        # ------------- persistent SBUF tensors -------------
        wh = persist.tile([128, KC, H], BF16, name="wh")     # stream [kp,kc,n]
        wdl = persist.tile([128, KC, H], BF16, name="wdl")   # Wp2 - Wh
        wp2 = persist.tile([128, KC, H], BF16, name="wp2")
        wtau = persist.tile([128, KC, H], BF16, name="wtau")
        wmem = persist.tile([128, KC, H], BF16, name="wmem")
        wout = persist.tile([128, KC, C], BF16, name="wout")
        i4b = persist.tile([4, 4], BF16, name="i4b")
        iota0 = persist.tile([128, D], FP32, name="iota0")    # 0..15
        iota2 = persist.tile([128, D], FP32, name="iota2")    # 2..17
        maskR = persist.tile([128, seq_len, BL], BF16, name="maskR")
        buf0 = persist.tile([128, CB, D], FP32, name="buf0")
        buf1 = persist.tile([128, CB, D], FP32, name="buf1")
        h0coll = persist.tile([128, KC, t_out, BL], BF16, name="h0coll")
        if not zero_bias:
            ones1b = persist.tile([1, BL], BF16, name="ones1b")
            btau_row = persist.tile([1, H], BF16, name="btau_row")
            bmem_row = persist.tile([1, H], BF16, name="bmem_row")
            cdec_row = persist.tile([1, H], BF16, name="cdec_row")
            b_out_r = persist.tile([128, C], FP32, name="b_out_r")

        # ------------- setup (scoped pools) -------------
        with tc.tile_pool(name="setup_ps", bufs=2, space="PSUM") as setup_ps, \
                tc.tile_pool(name="setup_sb", bufs=2) as setup_sb:
            # raw fp32 weight loads
            wh_d = setup_sb.tile([128, KC, H], FP32, name="wh_d")
            wpass_d = setup_sb.tile([128, KC, H], FP32, name="wpass_d")
            wtau_d = setup_sb.tile([128, KC, H], FP32, name="wtau_d")
            wmem_d = setup_sb.tile([128, KC, H], FP32, name="wmem_d")
            wx_d = setup_sb.tile([128, H], FP32, name="wx_d")
            wout_d = setup_sb.tile([128, KC, C], FP32, name="wout_d")
            nc.sync.dma_start(wh_d[:], dwin[:H].rearrange(
                "(kc kp) n -> kp kc n", kp=128))
            nc.sync.dma_start(wx_d[:], dwin[H:])
            nc.sync.dma_start(wpass_d[:], dwpass[:].rearrange(
                "(kc kp) n -> kp kc n", kp=128))
            nc.sync.dma_start(wtau_d[:], dwtau[:].rearrange(
                "(kc kp) n -> kp kc n", kp=128))
            nc.sync.dma_start(wmem_d[:], dwmem[:].rearrange(
                "(kc kp) n -> kp kc n", kp=128))
            nc.sync.dma_start(wout_d[:], dwout[:].rearrange(
                "(kc kp) n -> kp kc n", kp=128))
            # rounding copies
            wpass_r = setup_sb.tile([128, KC, H], FP32R, name="wpass_r")
            wx_r = setup_sb.tile([128, H], FP32R, name="wx_r")
            nc.vector.tensor_copy(wh[:], wh_d[:])
            nc.vector.tensor_copy(wtau[:], wtau_d[:])
            nc.vector.tensor_copy(wmem[:], wmem_d[:])
            nc.vector.tensor_copy(wpass_r[:], wpass_d[:])
            nc.vector.tensor_copy(wx_r[:], wx_d[:])
            nc.vector.tensor_copy(wout[:], wout_d[:])

            i4f = setup_sb.tile([4, 4], FP32, name="i4f")
            make_identity(nc, i4f[:])
            nc.vector.tensor_copy(i4b[:], i4f[:])
            id128 = setup_sb.tile([128, 128], FP32, name="id128")
            make_identity(nc, id128[:])

            iota16_i = setup_sb.tile([128, D], I32, name="iota16_i")
            nc.gpsimd.iota(iota16_i[:], pattern=[[1, D]], base=0,
                           channel_multiplier=0)
            nc.vector.tensor_copy(iota0[:], iota16_i[:])
            nc.gpsimd.iota(iota16_i[:], pattern=[[1, D]], base=2,
                           channel_multiplier=0)
            nc.vector.tensor_copy(iota2[:], iota16_i[:])

            # masks, replicated on every partition: maskR[p, t, b] = t < len[b]
            iota_t = setup_sb.tile([128, seq_len], I32, name="iota_t")
            nc.gpsimd.iota(iota_t[:], pattern=[[1, seq_len]], base=0,
                           channel_multiplier=0)
            lenR = setup_sb.tile([128, BL], I32, name="lenR")
            nc.sync.dma_start(
                lenR[:], dlen[:].unsqueeze(0).to_broadcast([128, BL]))
            mkR_i = setup_sb.tile([128, seq_len, BL], I32, name="mkR_i")
            nc.vector.tensor_tensor(
                out=mkR_i[:],
                in0=iota_t[:].unsqueeze(2).to_broadcast([128, seq_len, BL]),
                in1=lenR[:].unsqueeze(1).to_broadcast([128, seq_len, BL]),
                op=Op.is_lt)
            nc.vector.tensor_copy(maskR[:], mkR_i[:])

            # mask_bt[p, m], m = b*TPB + j, row r = 128*m + p (int mask
            # for copy_predicated)
            iota_bt = setup_sb.tile([128, TPB], I32, name="iota_bt")
            nc.gpsimd.iota(iota_bt[:], pattern=[[128, TPB]], base=0,
                           channel_multiplier=1)
            mk_bt_i = setup_sb.tile([128, BL, TPB], I32, name="mk_bt_i")
            nc.vector.tensor_tensor(
                out=mk_bt_i[:],
                in0=iota_bt[:].unsqueeze(1).to_broadcast([128, BL, TPB]),
                in1=lenR[:].unsqueeze(2).to_broadcast([128, BL, TPB]),
                op=Op.is_lt)

            # WhT / WxT via PE transposes (fp32 path), rounded to fp32r
            whT = setup_sb.tile([128, KC, H], FP32R, name="whT")
            wxT = setup_sb.tile([128, KC, I], FP32R, name="wxT")
            for jc in range(KC):
                for kc in range(KC):
                    pst = setup_ps.tile([128, 128], FP32, tag="setup_T")
                    nc.tensor.transpose(pst[:], wh_d[:, kc, bass.ts(jc, 128)],
                                        id128[:])
                    nc.vector.tensor_copy(whT[:, jc, bass.ts(kc, 128)],
                                          pst[:])
            for jc in range(KC):
                pst = setup_ps.tile([128, 128], FP32, tag="setup_T")
                nc.tensor.transpose(pst[:], wx_d[:, bass.ts(jc, 128)],
                                    id128[:])
                nc.vector.tensor_copy(wxT[:, jc, :], pst[:])

            # Wp2 = Wh @ W_pass ; Wxp = Wx @ W_pass ; dW = Wp2 - Wh
            wxp = setup_sb.tile([128, H], FP32R, name="wxp")
            wp2f = setup_sb.tile([128, KC, H], FP32, name="wp2f")
            for m in range(KC):
                psg = setup_ps.tile([128, H], FP32, tag="setup_G")
                for jc in range(KC):
                    nc.tensor.matmul(psg[:], whT[:, jc, bass.ts(m, 128)],
                                     wpass_r[:, jc, :],
                                     start=(jc == 0), stop=(jc == KC - 1))
                nc.vector.tensor_copy(wp2f[:, m, :], psg[:])
            nc.vector.tensor_copy(wp2[:], wp2f[:])
            psg = setup_ps.tile([128, H], FP32, tag="setup_G")
            for jc in range(KC):
                nc.tensor.matmul(psg[:], wxT[:, jc, :], wpass_r[:, jc, :],
                                 start=(jc == 0), stop=(jc == KC - 1))
            nc.vector.tensor_copy(wxp[:], psg[:])
            nc.vector.tensor_tensor(out=wdl[:], in0=wp2f[:],
                                    in1=wh_d[:], op=Op.subtract)

            # bias rows for the rank-1 bias matmuls + decode constant
            if not zero_bias:
                o1f = setup_sb.tile([1, BL], FP32, name="o1f")
                nc.vector.memset(o1f[:], 1.0)
                nc.vector.tensor_copy(ones1b[:], o1f[:])
                btd = setup_sb.tile([1, H], FP32, name="btd")
                bmd = setup_sb.tile([1, H], FP32, name="bmd")
                nc.sync.dma_start(btd[:], dbias["b_tau"][:].unsqueeze(0))
                nc.sync.dma_start(bmd[:], dbias["b_mem"][:].unsqueeze(0))
                nc.vector.tensor_copy(btau_row[:], btd[:])
                nc.vector.tensor_copy(bmem_row[:], bmd[:])
                nc.sync.dma_start(
                    b_out_r[:], dbias["b_out"][:].unsqueeze(0)
                    .to_broadcast([128, C]))
                # cdec = b_in @ W_pass + b_pass  (row vector)
                b_in_r = setup_sb.tile([128, H], FP32, name="b_in_r")
                nc.sync.dma_start(
                    b_in_r[:], dbias["b_in"][:].unsqueeze(0)
                    .to_broadcast([128, H]))
                binT = setup_sb.tile([128, KC, 1], FP32R, name="binT")
                binT_d = setup_sb.tile([128, KC, 1], FP32, name="binT_d")
                nc.sync.dma_start(
                    binT_d[:],
                    dbias["b_in"][:].rearrange("(c p) -> p c", p=128)
                    .unsqueeze(2))
                nc.vector.tensor_copy(binT[:], binT_d[:])
                psd = setup_ps.tile([1, H], FP32, tag="setup_D")
                for c in range(KC):
                    nc.tensor.matmul(psd[:], binT[:, c, :], wpass_r[:, c, :],
                                     start=(c == 0), stop=(c == KC - 1))
                bps = setup_sb.tile([1, H], FP32, name="bps")
                nc.sync.dma_start(bps[:], dbias["b_pass"][:].unsqueeze(0))
                nc.vector.tensor_tensor(out=cdec_row[:], in0=psd[:],
                                        in1=bps[:], op=Op.add)

            # x -> xT ; cx/cp/cmix precompute
            x_sb = setup_sb.tile([128, NMT, I], FP32, name="x_sb")
            xT = setup_sb.tile([128, NMT, 128], FP32R, name="xT")
            nc.sync.dma_start(
                x_sb[:],
                dx[:].rearrange("b t i -> (b t) i").rearrange(
                    "(m p) i -> p m i", p=128))
            for m in range(NMT):
                pst = setup_ps.tile([128, 128], FP32, tag="setup_T")
                nc.tensor.transpose(pst[:], x_sb[:, m, :], id128[:])
                nc.vector.tensor_copy(xT[:, m, :], pst[:])
            if not zero_bias:
                b_in_bc = b_in_r
                cdec_bc = setup_sb.tile([128, H], FP32, name="cdec_bc")
                nc.sync.dma_start(
                    cdec_bc[:], dbias["b_pass"][:].unsqueeze(0)
                    .to_broadcast([128, H]))
                # cdec broadcast = b_in@W_pass + b_pass on every partition:
                # recompute via per-partition copy from cdec_row is awkward;
                # use psd result broadcast through DRAM scratch.
                dcdec = nc.dram_tensor("cdec_scratch", [H], FP32)
                nc.sync.dma_start(dcdec[:], f32(cdec_row[:]).squeeze(0))
                cdec_full = setup_sb.tile([128, H], FP32, name="cdec_full")
                nc.sync.dma_start(
                    cdec_full[:], dcdec[:].unsqueeze(0).to_broadcast([128, H]))
            for m in range(NMT):
                ps1 = setup_ps.tile([128, H], FP32, tag="setup_G")
                nc.tensor.matmul(ps1[:], xT[:, m, :], wx_r[:],
                                 start=True, stop=True)
                cxt = setup_sb.tile([128, H], FP32R, tag="cxt", bufs=3)
                if zero_bias:
                    nc.vector.tensor_copy(cxt[:], ps1[:])
                else:
                    nc.vector.tensor_tensor(out=cxt[:], in0=ps1[:],
                                            in1=b_in_bc[:], op=Op.add)
                ps2 = setup_ps.tile([128, H], FP32, tag="setup_G")
                nc.tensor.matmul(ps2[:], xT[:, m, :], wxp[:],
                                 start=True, stop=True)
                cpt = setup_sb.tile([128, H], FP32R, tag="cpt", bufs=3)
                if zero_bias:
                    nc.vector.tensor_copy(cpt[:], ps2[:])
                else:
                    nc.vector.tensor_tensor(out=cpt[:], in0=ps2[:],
                                            in1=cdec_full[:], op=Op.add)
                nc.vector.copy_predicated(
                    out=f32(cxt[:]),
                    mask=mk_bt_i[:, m // TPB, m % TPB].unsqueeze(1)
                    .to_broadcast([128, H]),
                    data=f32(cpt[:]))
                cxb = setup_sb.tile([128, H], BF16, tag="cxb", bufs=3)
                nc.vector.tensor_copy(cxb[:], f32(cxt[:]))
                nc.sync.dma_start(dcmix[bass.ts(m, 128), :], cxb[:])

        # ------------- main recurrence -------------
        psum = ctx.enter_context(tc.tile_pool(name="mn_ps", bufs=1,
                                              space="PSUM"))
        psum2 = ctx.enter_context(tc.tile_pool(name="mn_ps2", bufs=2,
                                               space="PSUM"))
        loop_sb = ctx.enter_context(tc.tile_pool(name="mn_sb", bufs=2))
        dma_sb = ctx.enter_context(tc.tile_pool(name="mn_dma", bufs=4))

        cmix_v = dcmix[:].rearrange("(b t) n -> b t n", b=BL)
        bufs = [buf0, buf1]

        def _main_pass():
            h0 = loop_sb.tile([128, CB], BF16, tag="h0")
            nc.vector.memset(h0[:], 0.0)
            nc.gpsimd.memset(buf0[:], 0.0)

            deferred = []

            buf_idx = 0
            total_steps = seq_len + t_out
            for t in range(total_steps):
                is_enc = t < seq_len
                td = t - seq_len
                last = (t == total_steps - 1)

                if not is_enc:
                    nc.vector.tensor_copy(
                        h0coll[:, :, td, :],
                        h0[:].rearrange("p (c b) -> p c b", c=KC))
                    if last:
                        break

                # stationary for the masked dW stream
                if is_enc:
                    mT = maskR[:, t, :].unsqueeze(1).to_broadcast(
                        [128, KC, BL])
                    bst = loop_sb.tile([128, CB], BF16, tag="bst")
                    nc.vector.tensor_tensor(
                        out=bst[:].rearrange("p (c b) -> p c b", c=KC),
                        in0=h0[:].rearrange("p (c b) -> p c b", c=KC),
                        in1=mT, op=Op.mult)

                # deferred buffer update from the previous step (Pool-heavy)
                while deferred:
                    deferred.pop(0)()

                # ---- L1: 2 col groups, split by N-half ----
                # group 0 -> ps1[0:4, 0:NH], group 1 -> ps1[32:36, NH:H]
                ps1 = psum.tile([128, H], FP32, tag="ps1")
                h0_v = h0[:].rearrange("p (c b) -> p c b", c=KC)
                if is_enc:
                    cmix4 = dma_sb.tile([BL, H], BF16, tag="cmix4")
                    nc.sync.dma_start(cmix4[:], cmix_v[:, t, :])
                    bst_v = bst[:].rearrange("p (c b) -> p c b", c=KC)
                    # interleaved issue: g0/g1 alternate so both col groups
                    # stream concurrently
                    for c in range(KC):
                        nc.tensor.matmul(
                            ps1[0:BL, 0:NH], h0_v[:, c, :], wh[:, c, 0:NH],
                            start=(c == 0), stop=False,
                            tile_position=(0, 0))
                        nc.tensor.matmul(
                            ps1[32:32 + BL, NH:H], h0_v[:, c, :],
                            wh[:, c, NH:H],
                            start=(c == 0), stop=False,
                            tile_position=(0, 32))
                    for c in range(KC):
                        nc.tensor.matmul(
                            ps1[0:BL, 0:NH], bst_v[:, c, :], wdl[:, c, 0:NH],
                            start=False, stop=False,
                            tile_position=(0, 0))
                        nc.tensor.matmul(
                            ps1[32:32 + BL, NH:H], bst_v[:, c, :],
                            wdl[:, c, NH:H],
                            start=False, stop=False,
                            tile_position=(0, 32))
                    nc.tensor.matmul(
                        ps1[0:BL, 0:NH], i4b[:], cmix4[:, 0:NH],
                        start=False, stop=True, tile_position=(0, 0))
                    nc.tensor.matmul(
                        ps1[32:32 + BL, NH:H], i4b[:], cmix4[:, NH:H],
                        start=False, stop=True, tile_position=(0, 32))
                else:
                    for c in range(KC):
                        nc.tensor.matmul(
                            ps1[0:BL, 0:NH], h0_v[:, c, :], wp2[:, c, 0:NH],
                            start=(c == 0),
                            stop=(c == KC - 1 and zero_bias),
                            tile_position=(0, 0))
                        nc.tensor.matmul(
                            ps1[32:32 + BL, NH:H], h0_v[:, c, :],
                            wp2[:, c, NH:H],
                            start=(c == 0),
                            stop=(c == KC - 1 and zero_bias),
                            tile_position=(0, 32))
                    if not zero_bias:
                        nc.tensor.matmul(
                            ps1[0:BL, 0:NH], ones1b[:], cdec_row[:, 0:NH],
                            start=False, stop=True, tile_position=(0, 0))
                        nc.tensor.matmul(
                            ps1[32:32 + BL, NH:H], ones1b[:],
                            cdec_row[:, NH:H],
                            start=False, stop=True, tile_position=(0, 32))

                # h' PSUM -> SBUF: two halves on two engines
                h_sb = loop_sb.tile([BL, H], BF16, tag="h_sb")
                nc.vector.tensor_copy(h_sb[:, 0:NH], ps1[0:BL, 0:NH])
                nc.scalar.copy(h_sb[:, NH:H], ps1[32:32 + BL, NH:H])

                # T1: h' -> h'^T
                ps_t1 = psum2.tile([128, KC, BL], FP32, tag="ps_T1")
                for c in range(KC):
                    nc.tensor.matmul(ps_t1[:, c, :], h_sb[:, bass.ts(c, 128)],
                                     i4b[:], start=True, stop=True)
                hT = loop_sb.tile([128, CB], BF16, tag="hT")
                nc.vector.tensor_copy(
                    hT[:].rearrange("p (c b) -> p c b", c=KC), ps_t1[:])

                # ---- L2: tau in col group (0,64), mem in (0,96) ----
                ps2 = psum.tile([128, H], FP32, tag="ps2")
                hT_v = hT[:].rearrange("p (c b) -> p c b", c=KC)
                for half in range(2):
                    n0, n1 = half * NH, (half + 1) * NH
                    for c in range(KC):
                        nc.tensor.matmul(
                            ps2[64:64 + BL, n0:n1], hT_v[:, c, :],
                            wtau[:, c, n0:n1],
                            start=(c == 0),
                            stop=(c == KC - 1 and zero_bias),
                            tile_position=(0, 64))
                        nc.tensor.matmul(
                            ps2[96:96 + BL, n0:n1], hT_v[:, c, :],
                            wmem[:, c, n0:n1],
                            start=(c == 0),
                            stop=(c == KC - 1 and zero_bias),
                            tile_position=(0, 96))
                    if not zero_bias:
                        nc.tensor.matmul(
                            ps2[64:64 + BL, n0:n1], ones1b[:],
                            btau_row[:, n0:n1],
                            start=False, stop=True, tile_position=(0, 64))
                        nc.tensor.matmul(
                            ps2[96:96 + BL, n0:n1], ones1b[:],
                            bmem_row[:, n0:n1],
                            start=False, stop=True, tile_position=(0, 96))
                    # per-half PSUM->SBUF copies overlap the other half's
                    # stream: tau on ACT, mem on DVE
                tau_r = loop_sb.tile([BL, H], BF16, tag="tau_r")
                mem_r = loop_sb.tile([BL, H], BF16, tag="mem_r")
                for half in range(2):
                    n0, n1 = half * NH, (half + 1) * NH
                    nc.scalar.copy(tau_r[:, n0:n1], ps2[64:64 + BL, n0:n1])
                    nc.vector.tensor_copy(mem_r[:, n0:n1],
                                          ps2[96:96 + BL, n0:n1])

                # T2: tau_lin, mem_lin -> ^T (bf16 identity matmuls)
                ps_t2 = psum.tile([128, 2, KC, BL], FP32, tag="ps_T2")
                for c in range(KC):
                    nc.tensor.matmul(ps_t2[:, 0, c, :],
                                     tau_r[:, bass.ts(c, 128)], i4b[:],
                                     start=True, stop=True)
                    nc.tensor.matmul(ps_t2[:, 1, c, :],
                                     mem_r[:, bass.ts(c, 128)], i4b[:],
                                     start=True, stop=True)

                # sigmoid straight from PSUM + critical h0 update
                sig = loop_sb.tile([128, 2, CB], FP32, tag="sig")
                nc.scalar.activation(sig[:], ps_t2[:].rearrange(
                    "p a c b -> p a (c b)"), Sig)
                taup = loop_sb.tile([128, CB], FP32, tag="taup")
                nc.vector.tensor_scalar(out=taup[:], in0=sig[:, 0],
                                        scalar1=16.0, scalar2=1.0,
                                        op0=Op.mult, op1=Op.max)
                q = loop_sb.tile([128, CB], FP32, tag="q")
                nc.vector.tensor_tensor(
                    out=q[:], in0=sig[:, 1],
                    in1=ps_t1[:].rearrange("p c b -> p (c b)"), op=Op.mult)
                rtau = loop_sb.tile([128, CB], FP32, tag="rtau")
                nc.vector.reciprocal(out=rtau[:], in_=taup[:])
                t1 = loop_sb.tile([128, CB], FP32, tag="t1")
                nc.vector.tensor_tensor(out=t1[:], in0=q[:], in1=rtau[:],
                                        op=Op.mult)
                bcur, bnxt = bufs[buf_idx], bufs[buf_idx ^ 1]
                buf_idx ^= 1
                h0 = loop_sb.tile([128, CB], BF16, tag="h0")
                nc.vector.tensor_tensor(out=h0[:], in0=bcur[:, :, 0],
                                        in1=t1[:], op=Op.add)

                # deferred: buffer weights + shift-add (next step, on Pool)
                if t < total_steps - 2:
                    def make_deferred(taup=taup, q=q, bcur=bcur, bnxt=bnxt):
                        def emit():
                            # 1+|tau-d| = max(tau-(d-1), (d+1)-tau)
                            e0 = loop_sb.tile([128, CB, D], FP32, tag="e0")
                            nc.gpsimd.tensor_tensor(
                                out=e0[:],
                                in0=taup[:].unsqueeze(2).to_broadcast(
                                    [128, CB, D]),
                                in1=iota0[:].unsqueeze(1).to_broadcast(
                                    [128, CB, D]),
                                op=Op.subtract)
                            e1 = loop_sb.tile([128, CB, D], FP32, tag="e1")
                            nc.gpsimd.tensor_tensor(
                                out=e1[:],
                                in0=iota2[:].unsqueeze(1).to_broadcast(
                                    [128, CB, D]),
                                in1=taup[:].unsqueeze(2).to_broadcast(
                                    [128, CB, D]),
                                op=Op.subtract)
                            wa = loop_sb.tile([128, CB, D], FP32, tag="wa")
                            nc.vector.tensor_tensor(
                                out=wa[:], in0=e0[:], in1=e1[:], op=Op.max)
                            wr = loop_sb.tile([128, CB, D], FP32, tag="wr")
                            nc.vector.reciprocal_approx_fast(
                                out=wr[:], in_=wa[:])
                            wq = loop_sb.tile([128, CB, D], FP32, tag="wq")
                            nc.gpsimd.tensor_tensor(
                                out=wq[:], in0=wr[:],
                                in1=q[:].unsqueeze(2).to_broadcast(
                                    [128, CB, D]),
                                op=Op.mult)
                            nc.gpsimd.tensor_tensor(
                                out=bnxt[:, :, 0:D - 1], in0=bcur[:, :, 1:D],
                                in1=wq[:, :, 1:D], op=Op.add)
                            nc.gpsimd.memset(bnxt[:, :, D - 1], 0.0)
                        return emit
                    deferred.append(make_deferred())

            # batched output GEMM: out = h0coll @ W_out + b_out
            rows = t_out * BL
            for start in range(0, rows, 128):
                mrows = min(128, rows - start)
                t0, tn = start // BL, mrows // BL
                ps_o = psum.tile([mrows, C], FP32, tag="ps_o")
                for c in range(KC):
                    nc.tensor.matmul(
                        ps_o[:], h0coll[:, c, t0:t0 + tn, :], wout[:, c, :],
                        start=(c == 0), stop=(c == KC - 1))
                out_sb = loop_sb.tile([mrows, C], FP32, tag="out_sb")
                if zero_bias:
                    nc.vector.tensor_copy(out_sb[:], ps_o[:])
                else:
                    nc.vector.tensor_tensor(out=out_sb[:], in0=ps_o[:],
                                            in1=b_out_r[0:mrows, :],
                                            op=Op.add)
                for tt in range(tn):
                    nc.sync.dma_start(dout[:, t0 + tt, :],
                                      out_sb[bass.ts(tt, BL), :])

        for _rep in range(reps):
            _main_pass()

    nc.compile()
    return nc


_CACHE = {}


def _get_module(seq_len, t_out, zero_bias):
    key = (seq_len, t_out, zero_bias)
    if key not in _CACHE:
        _CACHE[key] = build(seq_len, t_out, zero_bias)
    return _CACHE[key]


def kernel(**inputs):
    x = np.ascontiguousarray(np.asarray(inputs["x"], dtype=np.float32))
    lengths = np.ascontiguousarray(
        np.asarray(inputs["lengths"]).astype(np.int32))
    t_out = int(inputs["out_lengths"])
    seq_len = x.shape[1]
    names = ["W_in", "W_pass", "W_tau", "W_mem", "W_out",
             "b_in", "b_pass", "b_tau", "b_mem", "b_out"]
    warrs = {n: np.ascontiguousarray(np.asarray(inputs[n], dtype=np.float32))
             for n in names}
    zero_bias = all(not np.any(warrs[n]) for n in
                    ["b_in", "b_pass", "b_tau", "b_mem", "b_out"])
    nc = _get_module(seq_len, t_out, zero_bias)

    from concourse import bass_utils
    in_maps = []
    for c in range(NCORES):
        sl = slice(c * BL, (c + 1) * BL)
        m = {"x": x[sl], "lengths": lengths[sl]}
        m.update(warrs)
        in_maps.append(m)
    res = bass_utils.run_bass_kernel_spmd(
        nc, in_maps, core_ids=list(range(NCORES)))
    out = np.concatenate([res.results[c]["out"] for c in range(NCORES)],
                         axis=0)
    return out


# revision 10
# speedup vs baseline: 559.4556x; 559.4556x over previous
"""DelayRNN Trainium2 kernel (v2).

Sharding (hardcoded from spec): data-parallel over batch. B=32 rows are
sharded 4-per-core across 8 NeuronCores; every core holds all weights in
SBUF and runs the full 256-step encode + 64-step decode recurrence for its
4 rows. No cross-core communication.

Math reformulation (validated vs reference):
  Wh = W_in[:H], Wx = W_in[H:]
  Wp2 = Wh @ W_pass ; dW = Wp2 - Wh        (folded, on device)
  cx_t = x_t @ Wx + b_in ;  cp_t = x_t @ (Wx@W_pass) + (b_in@W_pass+b_pass)
  cmix_t = m_t ? cp_t : cx_t               (precomputed batched, in DRAM)
  h'_t = h0@Wh + (m*h0)@dW + cmix_t        (PSUM-accumulated; cmix enters
                                            PSUM via a K=4 identity matmul)
  tau' = max(16*sigmoid(h'@W_tau + b_tau), 1); mem = sigmoid(h'@W_mem+b_mem)
  h0_next = buf[0] + (mem*h')/tau'
  nbuf[0:15] = buf[1:16] + q*r[1:16], q = mem*h', r_d = 1/(1+|tau'-d|)
Decode: h' = h0@Wp2 + cdec; out_t = h0@W_out + b_out batched at the end.

Performance structure (per step):
- L1/L2 matmul streams use 2x column tiling (tile_position col groups) so
  two weight streams flow through the PE concurrently via separate XBUSes.
  L1 splits by N-half: group (0,0) -> psum[0:4, 0:256] accumulates ALL
  half-0 terms (incl. cmix identity-MM), group (0,32) -> psum[32:36,
  256:512]. h' then needs only plain PSUM->SBUF copies (DVE half / ACT
  half, concurrent engines), no partial-sum adds.
- L2 runs tau in col group (0,64) and mem in (0,96) (bf16 weights, N=256
  halves) so the per-half psum copies overlap the other half's stream.
- tau/mem transposes (T2) are bf16; sigmoid reads the T2 PSUM directly.
- q reads the T1 PSUM directly so the h'-recurrence stays fp32r-exact;
  only the sigmoid paths see bf16 rounding.
- The buffer weight chain w = q/(1+|tau-d|) runs on Pool (gpsimd) +
  one DVE reciprocal_approx_fast, overlapped with the next step's L1.
fp32r moving streams need N>=256 for 1 cyc/row; bf16 has no such limit.
"""

import sys
import numpy as np

for _p in ("/opt/trn_rl_repo",):
    if _p not in sys.path:
        sys.path.append(_p)

from contextlib import ExitStack

import concourse.bass as bass
import concourse.tile as tile
from concourse import bacc, mybir
from concourse.masks import make_identity

FP32 = mybir.dt.float32
FP32R = mybir.dt.float32r
BF16 = mybir.dt.bfloat16
I32 = mybir.dt.int32

B, S, I, H, C = 32, 256, 128, 512, 64
T_OUT = 64
NCORES = 8
BL = B // NCORES        # 4 batch rows per core
KC = H // 128           # 4 k-chunks
D = 16                  # delay slots 1..16
CB = KC * BL            # 16 = chunks x batch (free size of ^T tiles)
NH = H // 2             # 256: N-half for L1/L2 streams

Sig = mybir.ActivationFunctionType.Sigmoid
Op = mybir.AluOpType


def f32(ap):
    return ap.bitcast(FP32)


def build(seq_len=S, t_out=T_OUT, zero_bias=True, reps=1):
    nc = bacc.Bacc("TRN2", target_bir_lowering=False, debug=False)

    # ---------------- DRAM I/O ----------------
    dx = nc.dram_tensor("x", [BL, seq_len, I], FP32, kind="ExternalInput")
    dlen = nc.dram_tensor("lengths", [BL], I32, kind="ExternalInput")
    dwin = nc.dram_tensor("W_in", [I + H, H], FP32, kind="ExternalInput")
    dwpass = nc.dram_tensor("W_pass", [H, H], FP32, kind="ExternalInput")
    dwtau = nc.dram_tensor("W_tau", [H, H], FP32, kind="ExternalInput")
    dwmem = nc.dram_tensor("W_mem", [H, H], FP32, kind="ExternalInput")
    dwout = nc.dram_tensor("W_out", [H, C], FP32, kind="ExternalInput")
    dbias = {}
    for nm, ln in [("b_in", H), ("b_pass", H), ("b_tau", H),
                   ("b_mem", H), ("b_out", C)]:
        dbias[nm] = nc.dram_tensor(nm, [ln], FP32, kind="ExternalInput")
    dout = nc.dram_tensor("out", [BL, t_out, C], FP32, kind="ExternalOutput")
    # internal DRAM scratch (fp32r so the per-step reload is pre-rounded)
    dcmix = nc.dram_tensor("cmix_scratch", [BL * seq_len, H], BF16)

    NROW = BL * seq_len            # bt rows
    NMT = NROW // 128              # row tiles for the cx/cp precompute
    TPB = seq_len // 128           # row tiles per batch row

    with tile.TileContext(nc) as tc, ExitStack() as ctx:
        persist = ctx.enter_context(tc.tile_pool(name="persist", bufs=1))

        # ------------- persistent SBUF tensors -------------
        wh = persist.tile([128, KC, H], BF16, name="wh")     # stream [kp,kc,n]
        wdl = persist.tile([128, KC, H], BF16, name="wdl")   # Wp2 - Wh
        wp2 = persist.tile([128, KC, H], BF16, name="wp2")
        wtau = persist.tile([128, KC, H], BF16, name="wtau")
        wmem = persist.tile([128, KC, H], BF16, name="wmem")
        wout = persist.tile([128, KC, C], BF16, name="wout")
        i4b = persist.tile([4, 4], BF16, name="i4b")
        iota0 = persist.tile([128, D], FP32, name="iota0")    # 0..15
        iota2 = persist.tile([128, D], FP32, name="iota2")    # 2..17
        maskR = persist.tile([128, seq_len, BL], BF16, name="maskR")
        buf0 = persist.tile([128, CB, D], FP32, name="buf0")
        buf1 = persist.tile([128, CB, D], FP32, name="buf1")
        h0coll = persist.tile([128, KC, t_out, BL], BF16, name="h0coll")
        if not zero_bias:
            ones1b = persist.tile([1, BL], BF16, name="ones1b")
            btau_row = persist.tile([1, H], BF16, name="btau_row")
            bmem_row = persist.tile([1, H], BF16, name="bmem_row")
            cdec_row = persist.tile([1, H], BF16, name="cdec_row")
            b_out_r = persist.tile([128, C], FP32, name="b_out_r")

        # ------------- setup (scoped pools) -------------
        with tc.tile_pool(name="setup_ps", bufs=2, space="PSUM") as setup_ps, \
                tc.tile_pool(name="setup_sb", bufs=2) as setup_sb:
            # raw fp32 weight loads
            wh_d = setup_sb.tile([128, KC, H], FP32, name="wh_d")
            wpass_d = setup_sb.tile([128, KC, H], FP32, name="wpass_d")
            wtau_d = setup_sb.tile([128, KC, H], FP32, name="wtau_d")
            wmem_d = setup_sb.tile([128, KC, H], FP32, name="wmem_d")
            wx_d = setup_sb.tile([128, H], FP32, name="wx_d")
            wout_d = setup_sb.tile([128, KC, C], FP32, name="wout_d")
            nc.sync.dma_start(wh_d[:], dwin[:H].rearrange(
                "(kc kp) n -> kp kc n", kp=128))
            nc.sync.dma_start(wx_d[:], dwin[H:])
            nc.sync.dma_start(wpass_d[:], dwpass[:].rearrange(
                "(kc kp) n -> kp kc n", kp=128))
            nc.sync.dma_start(wtau_d[:], dwtau[:].rearrange(
                "(kc kp) n -> kp kc n", kp=128))
            nc.sync.dma_start(wmem_d[:], dwmem[:].rearrange(
                "(kc kp) n -> kp kc n", kp=128))
            nc.sync.dma_start(wout_d[:], dwout[:].rearrange(
                "(kc kp) n -> kp kc n", kp=128))
            # rounding copies
            wpass_r = setup_sb.tile([128, KC, H], FP32R, name="wpass_r")
            wx_r = setup_sb.tile([128, H], FP32R, name="wx_r")
            nc.vector.tensor_copy(wh[:], wh_d[:])
            nc.vector.tensor_copy(wtau[:], wtau_d[:])
            nc.vector.tensor_copy(wmem[:], wmem_d[:])
            nc.vector.tensor_copy(wpass_r[:], wpass_d[:])
            nc.vector.tensor_copy(wx_r[:], wx_d[:])
            nc.vector.tensor_copy(wout[:], wout_d[:])

            i4f = setup_sb.tile([4, 4], FP32, name="i4f")
            make_identity(nc, i4f[:])
            nc.vector.tensor_copy(i4b[:], i4f[:])
            id128 = setup_sb.tile([128, 128], FP32, name="id128")
            make_identity(nc, id128[:])

            iota16_i = setup_sb.tile([128, D], I32, name="iota16_i")
            nc.gpsimd.iota(iota16_i[:], pattern=[[1, D]], base=0,
                           channel_multiplier=0)
            nc.vector.tensor_copy(iota0[:], iota16_i[:])
            nc.gpsimd.iota(iota16_i[:], pattern=[[1, D]], base=2,
                           channel_multiplier=0)
            nc.vector.tensor_copy(iota2[:], iota16_i[:])

            # masks, replicated on every partition: maskR[p, t, b] = t < len[b]
            iota_t = setup_sb.tile([128, seq_len], I32, name="iota_t")
            nc.gpsimd.iota(iota_t[:], pattern=[[1, seq_len]], base=0,
                           channel_multiplier=0)
            lenR = setup_sb.tile([128, BL], I32, name="lenR")
            nc.sync.dma_start(
                lenR[:], dlen[:].unsqueeze(0).to_broadcast([128, BL]))
            mkR_i = setup_sb.tile([128, seq_len, BL], I32, name="mkR_i")
            nc.vector.tensor_tensor(
                out=mkR_i[:],
                in0=iota_t[:].unsqueeze(2).to_broadcast([128, seq_len, BL]),
                in1=lenR[:].unsqueeze(1).to_broadcast([128, seq_len, BL]),
                op=Op.is_lt)
            nc.vector.tensor_copy(maskR[:], mkR_i[:])

            # mask_bt[p, m], m = b*TPB + j, row r = 128*m + p (int mask
            # for copy_predicated)
            iota_bt = setup_sb.tile([128, TPB], I32, name="iota_bt")
            nc.gpsimd.iota(iota_bt[:], pattern=[[128, TPB]], base=0,
                           channel_multiplier=1)
            mk_bt_i = setup_sb.tile([128, BL, TPB], I32, name="mk_bt_i")
            nc.vector.tensor_tensor(
                out=mk_bt_i[:],
                in0=iota_bt[:].unsqueeze(1).to_broadcast([128, BL, TPB]),
                in1=lenR[:].unsqueeze(2).to_broadcast([128, BL, TPB]),
                op=Op.is_lt)

            # WhT / WxT via PE transposes (fp32 path), rounded to fp32r
            whT = setup_sb.tile([128, KC, H], FP32R, name="whT")
            wxT = setup_sb.tile([128, KC, I], FP32R, name="wxT")
            for jc in range(KC):
                for kc in range(KC):
                    pst = setup_ps.tile([128, 128], FP32, tag="setup_T")
                    nc.tensor.transpose(pst[:], wh_d[:, kc, bass.ts(jc, 128)],
                                        id128[:])
                    nc.vector.tensor_copy(whT[:, jc, bass.ts(kc, 128)],
                                          pst[:])
            for jc in range(KC):
                pst = setup_ps.tile([128, 128], FP32, tag="setup_T")
                nc.tensor.transpose(pst[:], wx_d[:, bass.ts(jc, 128)],
                                    id128[:])
                nc.vector.tensor_copy(wxT[:, jc, :], pst[:])

            # Wp2 = Wh @ W_pass ; Wxp = Wx @ W_pass ; dW = Wp2 - Wh
            wxp = setup_sb.tile([128, H], FP32R, name="wxp")
            wp2f = setup_sb.tile([128, KC, H], FP32, name="wp2f")
            for m in range(KC):
                psg = setup_ps.tile([128, H], FP32, tag="setup_G")
                for jc in range(KC):
                    nc.tensor.matmul(psg[:], whT[:, jc, bass.ts(m, 128)],
                                     wpass_r[:, jc, :],
                                     start=(jc == 0), stop=(jc == KC - 1))
                nc.vector.tensor_copy(wp2f[:, m, :], psg[:])
            nc.vector.tensor_copy(wp2[:], wp2f[:])
            psg = setup_ps.tile([128, H], FP32, tag="setup_G")
            for jc in range(KC):
                nc.tensor.matmul(psg[:], wxT[:, jc, :], wpass_r[:, jc, :],
                                 start=(jc == 0), stop=(jc == KC - 1))
            nc.vector.tensor_copy(wxp[:], psg[:])
            nc.vector.tensor_tensor(out=wdl[:], in0=wp2f[:],
                                    in1=wh_d[:], op=Op.subtract)

            # bias rows for the rank-1 bias matmuls + decode constant
            if not zero_bias:
                o1f = setup_sb.tile([1, BL], FP32, name="o1f")
                nc.vector.memset(o1f[:], 1.0)
                nc.vector.tensor_copy(ones1b[:], o1f[:])
                btd = setup_sb.tile([1, H], FP32, name="btd")
                bmd = setup_sb.tile([1, H], FP32, name="bmd")
                nc.sync.dma_start(btd[:], dbias["b_tau"][:].unsqueeze(0))
                nc.sync.dma_start(bmd[:], dbias["b_mem"][:].unsqueeze(0))
                nc.vector.tensor_copy(btau_row[:], btd[:])
                nc.vector.tensor_copy(bmem_row[:], bmd[:])
                nc.sync.dma_start(
                    b_out_r[:], dbias["b_out"][:].unsqueeze(0)
                    .to_broadcast([128, C]))
                # cdec = b_in @ W_pass + b_pass  (row vector)
                b_in_r = setup_sb.tile([128, H], FP32, name="b_in_r")
                nc.sync.dma_start(
                    b_in_r[:], dbias["b_in"][:].unsqueeze(0)
                    .to_broadcast([128, H]))
                binT = setup_sb.tile([128, KC, 1], FP32R, name="binT")
                binT_d = setup_sb.tile([128, KC, 1], FP32, name="binT_d")
                nc.sync.dma_start(
                    binT_d[:],
                    dbias["b_in"][:].rearrange("(c p) -> p c", p=128)
                    .unsqueeze(2))
                nc.vector.tensor_copy(binT[:], binT_d[:])
                psd = setup_ps.tile([1, H], FP32, tag="setup_D")
                for c in range(KC):
                    nc.tensor.matmul(psd[:], binT[:, c, :], wpass_r[:, c, :],
                                     start=(c == 0), stop=(c == KC - 1))
                bps = setup_sb.tile([1, H], FP32, name="bps")
                nc.sync.dma_start(bps[:], dbias["b_pass"][:].unsqueeze(0))
                nc.vector.tensor_tensor(out=cdec_row[:], in0=psd[:],
                                        in1=bps[:], op=Op.add)

            # x -> xT ; cx/cp/cmix precompute
            x_sb = setup_sb.tile([128, NMT, I], FP32, name="x_sb")
            xT = setup_sb.tile([128, NMT, 128], FP32R, name="xT")
            nc.sync.dma_start(
                x_sb[:],
                dx[:].rearrange("b t i -> (b t) i").rearrange(
                    "(m p) i -> p m i", p=128))
            for m in range(NMT):
                pst = setup_ps.tile([128, 128], FP32, tag="setup_T")
                nc.tensor.transpose(pst[:], x_sb[:, m, :], id128[:])
                nc.vector.tensor_copy(xT[:, m, :], pst[:])
            if not zero_bias:
                b_in_bc = b_in_r
                cdec_bc = setup_sb.tile([128, H], FP32, name="cdec_bc")
                nc.sync.dma_start(
                    cdec_bc[:], dbias["b_pass"][:].unsqueeze(0)
                    .to_broadcast([128, H]))
                # cdec broadcast = b_in@W_pass + b_pass on every partition:
                # recompute via per-partition copy from cdec_row is awkward;
                # use psd result broadcast through DRAM scratch.
                dcdec = nc.dram_tensor("cdec_scratch", [H], FP32)
                nc.sync.dma_start(dcdec[:], f32(cdec_row[:]).squeeze(0))
                cdec_full = setup_sb.tile([128, H], FP32, name="cdec_full")
                nc.sync.dma_start(
                    cdec_full[:], dcdec[:].unsqueeze(0).to_broadcast([128, H]))
            for m in range(NMT):
                ps1 = setup_ps.tile([128, H], FP32, tag="setup_G")
                nc.tensor.matmul(ps1[:], xT[:, m, :], wx_r[:],
                                 start=True, stop=True)
                cxt = setup_sb.tile([128, H], FP32R, tag="cxt", bufs=3)
                if zero_bias:
                    nc.vector.tensor_copy(cxt[:], ps1[:])
                else:
                    nc.vector.tensor_tensor(out=cxt[:], in0=ps1[:],
                                            in1=b_in_bc[:], op=Op.add)
                ps2 = setup_ps.tile([128, H], FP32, tag="setup_G")
                nc.tensor.matmul(ps2[:], xT[:, m, :], wxp[:],
                                 start=True, stop=True)
                cpt = setup_sb.tile([128, H], FP32R, tag="cpt", bufs=3)
                if zero_bias:
                    nc.vector.tensor_copy(cpt[:], ps2[:])
                else:
                    nc.vector.tensor_tensor(out=cpt[:], in0=ps2[:],
                                            in1=cdec_full[:], op=Op.add)
                nc.vector.copy_predicated(
                    out=f32(cxt[:]),
                    mask=mk_bt_i[:, m // TPB, m % TPB].unsqueeze(1)
                    .to_broadcast([128, H]),
                    data=f32(cpt[:]))
                cxb = setup_sb.tile([128, H], BF16, tag="cxb", bufs=3)
                nc.vector.tensor_copy(cxb[:], f32(cxt[:]))
                nc.sync.dma_start(dcmix[bass.ts(m, 128), :], cxb[:])

        # ------------- main recurrence -------------
        psum = ctx.enter_context(tc.tile_pool(name="mn_ps", bufs=1,
                                              space="PSUM"))
        psum2 = ctx.enter_context(tc.tile_pool(name="mn_ps2", bufs=2,
                                               space="PSUM"))
        loop_sb = ctx.enter_context(tc.tile_pool(name="mn_sb", bufs=2))
        dma_sb = ctx.enter_context(tc.tile_pool(name="mn_dma", bufs=4))

        cmix_v = dcmix[:].rearrange("(b t) n -> b t n", b=BL)
        bufs = [buf0, buf1]

        def _main_pass():
            h0 = loop_sb.tile([128, CB], BF16, tag="h0")
            nc.vector.memset(h0[:], 0.0)
            nc.gpsimd.memset(buf0[:], 0.0)

            deferred = []

            buf_idx = 0
            total_steps = seq_len + t_out
            for t in range(total_steps):
                is_enc = t < seq_len
                td = t - seq_len
                last = (t == total_steps - 1)

                if not is_enc:
                    nc.vector.tensor_copy(
                        h0coll[:, :, td, :],
                        h0[:].rearrange("p (c b) -> p c b", c=KC))
                    if last:
                        break

                # stationary for the masked dW stream
                if is_enc:
                    mT = maskR[:, t, :].unsqueeze(1).to_broadcast(
                        [128, KC, BL])
                    bst = loop_sb.tile([128, CB], BF16, tag="bst")
                    nc.vector.tensor_tensor(
                        out=bst[:].rearrange("p (c b) -> p c b", c=KC),
                        in0=h0[:].rearrange("p (c b) -> p c b", c=KC),
                        in1=mT, op=Op.mult)

                # deferred buffer update from the previous step (Pool-heavy)
                while deferred:
                    deferred.pop(0)()

                # ---- L1: 2 col groups, split by N-half ----
                # group 0 -> ps1[0:4, 0:NH], group 1 -> ps1[32:36, NH:H]
                ps1 = psum.tile([128, H], FP32, tag="ps1")
                h0_v = h0[:].rearrange("p (c b) -> p c b", c=KC)
                if is_enc:
                    cmix4 = dma_sb.tile([BL, H], BF16, tag="cmix4")
                    nc.sync.dma_start(cmix4[:], cmix_v[:, t, :])
                    bst_v = bst[:].rearrange("p (c b) -> p c b", c=KC)
                    # interleaved issue: g0/g1 alternate so both col groups
                    # stream concurrently
                    for c in range(KC):
                        nc.tensor.matmul(
                            ps1[0:BL, 0:NH], h0_v[:, c, :], wh[:, c, 0:NH],
                            start=(c == 0), stop=False,
                            tile_position=(0, 0))
                        nc.tensor.matmul(
                            ps1[32:32 + BL, NH:H], h0_v[:, c, :],
                            wh[:, c, NH:H],
                            start=(c == 0), stop=False,
                            tile_position=(0, 32))
                    for c in range(KC):
                        nc.tensor.matmul(
                            ps1[0:BL, 0:NH], bst_v[:, c, :], wdl[:, c, 0:NH],
                            start=False, stop=False,
                            tile_position=(0, 0))
                        nc.tensor.matmul(
                            ps1[32:32 + BL, NH:H], bst_v[:, c, :],
                            wdl[:, c, NH:H],
                            start=False, stop=False,
                            tile_position=(0, 32))
                    nc.tensor.matmul(
                        ps1[0:BL, 0:NH], i4b[:], cmix4[:, 0:NH],
                        start=False, stop=True, tile_position=(0, 0))
                    nc.tensor.matmul(
                        ps1[32:32 + BL, NH:H], i4b[:], cmix4[:, NH:H],
                        start=False, stop=True, tile_position=(0, 32))
                else:
                    for c in range(KC):
                        nc.tensor.matmul(
                            ps1[0:BL, 0:NH], h0_v[:, c, :], wp2[:, c, 0:NH],
                            start=(c == 0),
                            stop=(c == KC - 1 and zero_bias),
                            tile_position=(0, 0))
                        nc.tensor.matmul(
                            ps1[32:32 + BL, NH:H], h0_v[:, c, :],
                            wp2[:, c, NH:H],
                            start=(c == 0),
                            stop=(c == KC - 1 and zero_bias),
                            tile_position=(0, 32))
                    if not zero_bias:
                        nc.tensor.matmul(
                            ps1[0:BL, 0:NH], ones1b[:], cdec_row[:, 0:NH],
                            start=False, stop=True, tile_position=(0, 0))
                        nc.tensor.matmul(
                            ps1[32:32 + BL, NH:H], ones1b[:],
                            cdec_row[:, NH:H],
                            start=False, stop=True, tile_position=(0, 32))

                # h' PSUM -> SBUF: two halves on two engines
                h_sb = loop_sb.tile([BL, H], BF16, tag="h_sb")
                nc.vector.tensor_copy(h_sb[:, 0:NH], ps1[0:BL, 0:NH])
                nc.scalar.copy(h_sb[:, NH:H], ps1[32:32 + BL, NH:H])

                # T1: h' -> h'^T
                ps_t1 = psum2.tile([128, KC, BL], FP32, tag="ps_T1")
                for c in range(KC):
                    nc.tensor.matmul(ps_t1[:, c, :], h_sb[:, bass.ts(c, 128)],
                                     i4b[:], start=True, stop=True)
                hT = loop_sb.tile([128, CB], BF16, tag="hT")
                hT_c = hT[:].rearrange("p (c b) -> p c b", c=KC)
                for c in range(KC):
                    nc.vector.tensor_copy(hT_c[:, c, :], ps_t1[:, c, :])

                # ---- L2: tau in col group (0,64), mem in (0,96) ----
                ps2 = psum.tile([128, H], FP32, tag="ps2")
                hT_v = hT[:].rearrange("p (c b) -> p c b", c=KC)
                for half in range(2):
                    n0, n1 = half * NH, (half + 1) * NH
                    for c in range(KC):
                        nc.tensor.matmul(
                            ps2[64:64 + BL, n0:n1], hT_v[:, c, :],
                            wtau[:, c, n0:n1],
                            start=(c == 0),
                            stop=(c == KC - 1 and zero_bias),
                            tile_position=(0, 64))
                        nc.tensor.matmul(
                            ps2[96:96 + BL, n0:n1], hT_v[:, c, :],
                            wmem[:, c, n0:n1],
                            start=(c == 0),
                            stop=(c == KC - 1 and zero_bias),
                            tile_position=(0, 96))
                    if not zero_bias:
                        nc.tensor.matmul(
                            ps2[64:64 + BL, n0:n1], ones1b[:],
                            btau_row[:, n0:n1],
                            start=False, stop=True, tile_position=(0, 64))
                        nc.tensor.matmul(
                            ps2[96:96 + BL, n0:n1], ones1b[:],
                            bmem_row[:, n0:n1],
                            start=False, stop=True, tile_position=(0, 96))
                    # per-half PSUM->SBUF copies overlap the other half's
                    # stream: tau on ACT, mem on DVE
                tau_r = loop_sb.tile([BL, H], BF16, tag="tau_r")
                mem_r = loop_sb.tile([BL, H], BF16, tag="mem_r")
                for half in range(2):
                    n0, n1 = half * NH, (half + 1) * NH
                    nc.scalar.copy(tau_r[:, n0:n1], ps2[64:64 + BL, n0:n1])
                    nc.vector.tensor_copy(mem_r[:, n0:n1],
                                          ps2[96:96 + BL, n0:n1])

                # T2: tau_lin, mem_lin -> ^T (bf16 identity matmuls);
                # tau first so its sigmoid overlaps the mem transposes
                ps_t2 = psum.tile([128, 2, KC, BL], FP32, tag="ps_T2")
                for c in range(KC):
                    nc.tensor.matmul(ps_t2[:, 0, c, :],
                                     tau_r[:, bass.ts(c, 128)], i4b[:],
                                     start=True, stop=True)
                sig = loop_sb.tile([128, 2, CB], FP32, tag="sig")
                nc.scalar.activation(sig[:, 0], ps_t2[:, 0].rearrange(
                    "p c b -> p (c b)"), Sig)
                taup = loop_sb.tile([128, CB], FP32, tag="taup")
                nc.vector.tensor_scalar(out=taup[:], in0=sig[:, 0],
                                        scalar1=16.0, scalar2=1.0,
                                        op0=Op.mult, op1=Op.max)
                for c in range(KC):
                    nc.tensor.matmul(ps_t2[:, 1, c, :],
                                     mem_r[:, bass.ts(c, 128)], i4b[:],
                                     start=True, stop=True)
                nc.scalar.activation(sig[:, 1], ps_t2[:, 1].rearrange(
                    "p c b -> p (c b)"), Sig)
                q = loop_sb.tile([128, CB], FP32, tag="q")
                nc.vector.tensor_tensor(
                    out=q[:], in0=sig[:, 1],
                    in1=ps_t1[:].rearrange("p c b -> p (c b)"), op=Op.mult)
                rtau = loop_sb.tile([128, CB], FP32, tag="rtau")
                nc.vector.reciprocal(out=rtau[:], in_=taup[:])
                t1 = loop_sb.tile([128, CB], FP32, tag="t1")
                nc.vector.tensor_tensor(out=t1[:], in0=q[:], in1=rtau[:],
                                        op=Op.mult)
                bcur, bnxt = bufs[buf_idx], bufs[buf_idx ^ 1]
                buf_idx ^= 1
                h0 = loop_sb.tile([128, CB], BF16, tag="h0")
                nc.vector.tensor_tensor(out=h0[:], in0=bcur[:, :, 0],
                                        in1=t1[:], op=Op.add)

                # deferred: buffer weights + shift-add (next step, on Pool)
                if t < total_steps - 2:
                    def make_deferred(taup=taup, q=q, bcur=bcur, bnxt=bnxt):
                        def emit():
                            # 1+|tau-d| = max(tau-(d-1), (d+1)-tau)
                            e0 = loop_sb.tile([128, CB, D], FP32, tag="e0")
                            nc.gpsimd.tensor_tensor(
                                out=e0[:],
                                in0=taup[:].unsqueeze(2).to_broadcast(
                                    [128, CB, D]),
                                in1=iota0[:].unsqueeze(1).to_broadcast(
                                    [128, CB, D]),
                                op=Op.subtract)
                            e1 = loop_sb.tile([128, CB, D], FP32, tag="e1")
                            nc.gpsimd.tensor_tensor(
                                out=e1[:],
                                in0=iota2[:].unsqueeze(1).to_broadcast(
                                    [128, CB, D]),
                                in1=taup[:].unsqueeze(2).to_broadcast(
                                    [128, CB, D]),
                                op=Op.subtract)
                            wa = loop_sb.tile([128, CB, D], FP32, tag="wa")
                            nc.vector.tensor_tensor(
                                out=wa[:], in0=e0[:], in1=e1[:], op=Op.max)
                            wr = loop_sb.tile([128, CB, D], FP32, tag="wr")
                            nc.vector.reciprocal_approx_fast(
                                out=wr[:], in_=wa[:])
                            wq = loop_sb.tile([128, CB, D], FP32, tag="wq")
                            nc.gpsimd.tensor_tensor(
                                out=wq[:], in0=wr[:],
                                in1=q[:].unsqueeze(2).to_broadcast(
                                    [128, CB, D]),
                                op=Op.mult)
                            nc.gpsimd.tensor_tensor(
                                out=bnxt[:, :, 0:D - 1], in0=bcur[:, :, 1:D],
                                in1=wq[:, :, 1:D], op=Op.add)
                            nc.gpsimd.memset(bnxt[:, :, D - 1], 0.0)
                        return emit
                    deferred.append(make_deferred())

            # batched output GEMM: out = h0coll @ W_out + b_out
            rows = t_out * BL
            for start in range(0, rows, 128):
                mrows = min(128, rows - start)
                t0, tn = start // BL, mrows // BL
                ps_o = psum.tile([mrows, C], FP32, tag="ps_o")
                for c in range(KC):
                    nc.tensor.matmul(
                        ps_o[:], h0coll[:, c, t0:t0 + tn, :], wout[:, c, :],
                        start=(c == 0), stop=(c == KC - 1))
                out_sb = loop_sb.tile([mrows, C], FP32, tag="out_sb")
                if zero_bias:
                    nc.vector.tensor_copy(out_sb[:], ps_o[:])
                else:
                    nc.vector.tensor_tensor(out=out_sb[:], in0=ps_o[:],
                                            in1=b_out_r[0:mrows, :],
                                            op=Op.add)
                for tt in range(tn):
                    nc.sync.dma_start(dout[:, t0 + tt, :],
                                      out_sb[bass.ts(tt, BL), :])

        for _rep in range(reps):
            _main_pass()

    nc.compile()
    return nc


_CACHE = {}


def _get_module(seq_len, t_out, zero_bias):
    key = (seq_len, t_out, zero_bias)
    if key not in _CACHE:
        _CACHE[key] = build(seq_len, t_out, zero_bias)
    return _CACHE[key]


def kernel(**inputs):
    x = np.ascontiguousarray(np.asarray(inputs["x"], dtype=np.float32))
    lengths = np.ascontiguousarray(
        np.asarray(inputs["lengths"]).astype(np.int32))
    t_out = int(inputs["out_lengths"])
    seq_len = x.shape[1]
    names = ["W_in", "W_pass", "W_tau", "W_mem", "W_out",
             "b_in", "b_pass", "b_tau", "b_mem", "b_out"]
    warrs = {n: np.ascontiguousarray(np.asarray(inputs[n], dtype=np.float32))
             for n in names}
    zero_bias = all(not np.any(warrs[n]) for n in
                    ["b_in", "b_pass", "b_tau", "b_mem", "b_out"])
    nc = _get_module(seq_len, t_out, zero_bias)

    from concourse import bass_utils
    in_maps = []
    for c in range(NCORES):
        sl = slice(c * BL, (c + 1) * BL)
        m = {"x": x[sl], "lengths": lengths[sl]}
        m.update(warrs)
        in_maps.append(m)
    res = bass_utils.run_bass_kernel_spmd(
        nc, in_maps, core_ids=list(range(NCORES)))
    out = np.concatenate([res.results[c]["out"] for c in range(NCORES)],
                         axis=0)
    return out


# revision 11
# speedup vs baseline: 864.4334x; 1.5451x over previous
"""DelayRNN Trainium2 kernel (v2).

Sharding (hardcoded from spec): data-parallel over batch. B=32 rows are
sharded 4-per-core across 8 NeuronCores; every core holds all weights in
SBUF and runs the full 256-step encode + 64-step decode recurrence for its
4 rows. No cross-core communication.

Math reformulation (validated vs reference):
  Wh = W_in[:H], Wx = W_in[H:]
  Wp2 = Wh @ W_pass ; dW = Wp2 - Wh        (folded, on device)
  cx_t = x_t @ Wx + b_in ;  cp_t = x_t @ (Wx@W_pass) + (b_in@W_pass+b_pass)
  cmix_t = m_t ? cp_t : cx_t               (precomputed batched, in DRAM)
  h'_t = h0@Wh + (m*h0)@dW + cmix_t        (PSUM-accumulated; cmix enters
                                            PSUM via a K=4 identity matmul)
  tau' = max(16*sigmoid(h'@W_tau + b_tau), 1); mem = sigmoid(h'@W_mem+b_mem)
  h0_next = buf[0] + (mem*h')/tau'
  nbuf[0:15] = buf[1:16] + q*r[1:16], q = mem*h', r_d = 1/(1+|tau'-d|)
Decode: h' = h0@Wp2 + cdec; out_t = h0@W_out + b_out batched at the end.

Performance structure (per step):
- L1/L2 matmul streams use 2x column tiling (tile_position col groups) so
  two weight streams flow through the PE concurrently via separate XBUSes.
  L1 splits by N-half: group (0,0) -> psum[0:4, 0:256] accumulates ALL
  half-0 terms (incl. cmix identity-MM), group (0,32) -> psum[32:36,
  256:512]. h' then needs only plain PSUM->SBUF copies (DVE half / ACT
  half, concurrent engines), no partial-sum adds.
- L2 runs tau in col group (0,64) and mem in (0,96) (bf16 weights, N=256
  halves) so the per-half psum copies overlap the other half's stream.
- tau/mem transposes (T2) are bf16; sigmoid reads the T2 PSUM directly.
- q reads the T1 PSUM directly so the h'-recurrence stays fp32r-exact;
  only the sigmoid paths see bf16 rounding.
- The buffer weight chain w = q/(1+|tau-d|) runs on Pool (gpsimd) +
  one DVE reciprocal_approx_fast, overlapped with the next step's L1.
fp32r moving streams need N>=256 for 1 cyc/row; bf16 has no such limit.
"""

import sys
import numpy as np

for _p in ("/opt/trn_rl_repo",):
    if _p not in sys.path:
        sys.path.append(_p)

from contextlib import ExitStack

import concourse.bass as bass
import concourse.tile as tile
from concourse import bacc, mybir
from concourse.masks import make_identity

FP32 = mybir.dt.float32
FP32R = mybir.dt.float32r
BF16 = mybir.dt.bfloat16
I32 = mybir.dt.int32

B, S, I, H, C = 32, 256, 128, 512, 64
T_OUT = 64
NCORES = 8
BL = B // NCORES        # 4 batch rows per core
KC = H // 128           # 4 k-chunks
D = 16                  # delay slots 1..16
CB = KC * BL            # 16 = chunks x batch (free size of ^T tiles)
NH = H // 2             # 256: N-half for L1/L2 streams

Sig = mybir.ActivationFunctionType.Sigmoid
Op = mybir.AluOpType


def f32(ap):
    return ap.bitcast(FP32)


def build(seq_len=S, t_out=T_OUT, zero_bias=True, reps=1):
    nc = bacc.Bacc("TRN2", target_bir_lowering=False, debug=False)

    # ---------------- DRAM I/O ----------------
    dx = nc.dram_tensor("x", [BL, seq_len, I], FP32, kind="ExternalInput")
    dlen = nc.dram_tensor("lengths", [BL], I32, kind="ExternalInput")
    dwin = nc.dram_tensor("W_in", [I + H, H], FP32, kind="ExternalInput")
    dwpass = nc.dram_tensor("W_pass", [H, H], FP32, kind="ExternalInput")
    dwtau = nc.dram_tensor("W_tau", [H, H], FP32, kind="ExternalInput")
    dwmem = nc.dram_tensor("W_mem", [H, H], FP32, kind="ExternalInput")
    dwout = nc.dram_tensor("W_out", [H, C], FP32, kind="ExternalInput")
    dbias = {}
    for nm, ln in [("b_in", H), ("b_pass", H), ("b_tau", H),
                   ("b_mem", H), ("b_out", C)]:
        dbias[nm] = nc.dram_tensor(nm, [ln], FP32, kind="ExternalInput")
    dout = nc.dram_tensor("out", [BL, t_out, C], FP32, kind="ExternalOutput")
    # internal DRAM scratch (fp32r so the per-step reload is pre-rounded)
    dcmix = nc.dram_tensor("cmix_scratch", [BL * seq_len, H], BF16)

    NROW = BL * seq_len            # bt rows
    NMT = NROW // 128              # row tiles for the cx/cp precompute
    TPB = seq_len // 128           # row tiles per batch row

    with tile.TileContext(nc) as tc, ExitStack() as ctx:
        persist = ctx.enter_context(tc.tile_pool(name="persist", bufs=1))

        # ------------- persistent SBUF tensors -------------
        wh = persist.tile([128, KC, H], BF16, name="wh")     # stream [kp,kc,n]
        wdl = persist.tile([128, KC, H], BF16, name="wdl")   # Wp2 - Wh
        wp2 = persist.tile([128, KC, H], BF16, name="wp2")
        wtau = persist.tile([128, KC, H], BF16, name="wtau")
        wmem = persist.tile([128, KC, H], BF16, name="wmem")
        wout = persist.tile([128, KC, C], BF16, name="wout")
        i4b = persist.tile([4, 4], BF16, name="i4b")
        iota0 = persist.tile([128, D], FP32, name="iota0")    # 0..15
        iota2 = persist.tile([128, D], FP32, name="iota2")    # 2..17
        maskR = persist.tile([128, seq_len, BL], BF16, name="maskR")
        buf0 = persist.tile([128, CB, D], FP32, name="buf0")
        buf1 = persist.tile([128, CB, D], FP32, name="buf1")
        h0coll = persist.tile([128, KC, t_out, BL], BF16, name="h0coll")
        if not zero_bias:
            ones1b = persist.tile([1, BL], BF16, name="ones1b")
            btau_row = persist.tile([1, H], BF16, name="btau_row")
            bmem_row = persist.tile([1, H], BF16, name="bmem_row")
            cdec_row = persist.tile([1, H], BF16, name="cdec_row")
            b_out_r = persist.tile([128, C], FP32, name="b_out_r")

        # ------------- setup (scoped pools) -------------
        with tc.tile_pool(name="setup_ps", bufs=2, space="PSUM") as setup_ps, \
                tc.tile_pool(name="setup_sb", bufs=2) as setup_sb:
            # raw fp32 weight loads
            wh_d = setup_sb.tile([128, KC, H], FP32, name="wh_d")
            wpass_d = setup_sb.tile([128, KC, H], FP32, name="wpass_d")
            wtau_d = setup_sb.tile([128, KC, H], FP32, name="wtau_d")
            wmem_d = setup_sb.tile([128, KC, H], FP32, name="wmem_d")
            wx_d = setup_sb.tile([128, H], FP32, name="wx_d")
            wout_d = setup_sb.tile([128, KC, C], FP32, name="wout_d")
            nc.sync.dma_start(wh_d[:], dwin[:H].rearrange(
                "(kc kp) n -> kp kc n", kp=128))
            nc.sync.dma_start(wx_d[:], dwin[H:])
            nc.sync.dma_start(wpass_d[:], dwpass[:].rearrange(
                "(kc kp) n -> kp kc n", kp=128))
            nc.sync.dma_start(wtau_d[:], dwtau[:].rearrange(
                "(kc kp) n -> kp kc n", kp=128))
            nc.sync.dma_start(wmem_d[:], dwmem[:].rearrange(
                "(kc kp) n -> kp kc n", kp=128))
            nc.sync.dma_start(wout_d[:], dwout[:].rearrange(
                "(kc kp) n -> kp kc n", kp=128))
            # rounding copies
            wpass_r = setup_sb.tile([128, KC, H], FP32R, name="wpass_r")
            wx_r = setup_sb.tile([128, H], FP32R, name="wx_r")
            nc.vector.tensor_copy(wh[:], wh_d[:])
            nc.vector.tensor_copy(wtau[:], wtau_d[:])
            nc.vector.tensor_copy(wmem[:], wmem_d[:])
            nc.vector.tensor_copy(wpass_r[:], wpass_d[:])
            nc.vector.tensor_copy(wx_r[:], wx_d[:])
            nc.vector.tensor_copy(wout[:], wout_d[:])

            i4f = setup_sb.tile([4, 4], FP32, name="i4f")
            make_identity(nc, i4f[:])
            nc.vector.tensor_copy(i4b[:], i4f[:])
            id128 = setup_sb.tile([128, 128], FP32, name="id128")
            make_identity(nc, id128[:])

            iota16_i = setup_sb.tile([128, D], I32, name="iota16_i")
            nc.gpsimd.iota(iota16_i[:], pattern=[[1, D]], base=0,
                           channel_multiplier=0)
            nc.vector.tensor_copy(iota0[:], iota16_i[:])
            nc.gpsimd.iota(iota16_i[:], pattern=[[1, D]], base=2,
                           channel_multiplier=0)
            nc.vector.tensor_copy(iota2[:], iota16_i[:])

            # masks, replicated on every partition: maskR[p, t, b] = t < len[b]
            iota_t = setup_sb.tile([128, seq_len], I32, name="iota_t")
            nc.gpsimd.iota(iota_t[:], pattern=[[1, seq_len]], base=0,
                           channel_multiplier=0)
            lenR = setup_sb.tile([128, BL], I32, name="lenR")
            nc.sync.dma_start(
                lenR[:], dlen[:].unsqueeze(0).to_broadcast([128, BL]))
            mkR_i = setup_sb.tile([128, seq_len, BL], I32, name="mkR_i")
            nc.vector.tensor_tensor(
                out=mkR_i[:],
                in0=iota_t[:].unsqueeze(2).to_broadcast([128, seq_len, BL]),
                in1=lenR[:].unsqueeze(1).to_broadcast([128, seq_len, BL]),
                op=Op.is_lt)
            nc.vector.tensor_copy(maskR[:], mkR_i[:])

            # mask_bt[p, m], m = b*TPB + j, row r = 128*m + p (int mask
            # for copy_predicated)
            iota_bt = setup_sb.tile([128, TPB], I32, name="iota_bt")
            nc.gpsimd.iota(iota_bt[:], pattern=[[128, TPB]], base=0,
                           channel_multiplier=1)
            mk_bt_i = setup_sb.tile([128, BL, TPB], I32, name="mk_bt_i")
            nc.vector.tensor_tensor(
                out=mk_bt_i[:],
                in0=iota_bt[:].unsqueeze(1).to_broadcast([128, BL, TPB]),
                in1=lenR[:].unsqueeze(2).to_broadcast([128, BL, TPB]),
                op=Op.is_lt)

            # WhT / WxT via PE transposes (fp32 path), rounded to fp32r
            whT = setup_sb.tile([128, KC, H], FP32R, name="whT")
            wxT = setup_sb.tile([128, KC, I], FP32R, name="wxT")
            for jc in range(KC):
                for kc in range(KC):
                    pst = setup_ps.tile([128, 128], FP32, tag="setup_T")
                    nc.tensor.transpose(pst[:], wh_d[:, kc, bass.ts(jc, 128)],
                                        id128[:])
                    nc.vector.tensor_copy(whT[:, jc, bass.ts(kc, 128)],
                                          pst[:])
            for jc in range(KC):
                pst = setup_ps.tile([128, 128], FP32, tag="setup_T")
                nc.tensor.transpose(pst[:], wx_d[:, bass.ts(jc, 128)],
                                    id128[:])
                nc.vector.tensor_copy(wxT[:, jc, :], pst[:])

            # Wp2 = Wh @ W_pass ; Wxp = Wx @ W_pass ; dW = Wp2 - Wh
            wxp = setup_sb.tile([128, H], FP32R, name="wxp")
            wp2f = setup_sb.tile([128, KC, H], FP32, name="wp2f")
            for m in range(KC):
                psg = setup_ps.tile([128, H], FP32, tag="setup_G")
                for jc in range(KC):
                    nc.tensor.matmul(psg[:], whT[:, jc, bass.ts(m, 128)],
                                     wpass_r[:, jc, :],
                                     start=(jc == 0), stop=(jc == KC - 1))
                nc.vector.tensor_copy(wp2f[:, m, :], psg[:])
            nc.vector.tensor_copy(wp2[:], wp2f[:])
            psg = setup_ps.tile([128, H], FP32, tag="setup_G")
            for jc in range(KC):
                nc.tensor.matmul(psg[:], wxT[:, jc, :], wpass_r[:, jc, :],
                                 start=(jc == 0), stop=(jc == KC - 1))
            nc.vector.tensor_copy(wxp[:], psg[:])
            nc.vector.tensor_tensor(out=wdl[:], in0=wp2f[:],
                                    in1=wh_d[:], op=Op.subtract)

            # bias rows for the rank-1 bias matmuls + decode constant
            if not zero_bias:
                o1f = setup_sb.tile([1, BL], FP32, name="o1f")
                nc.vector.memset(o1f[:], 1.0)
                nc.vector.tensor_copy(ones1b[:], o1f[:])
                btd = setup_sb.tile([1, H], FP32, name="btd")
                bmd = setup_sb.tile([1, H], FP32, name="bmd")
                nc.sync.dma_start(btd[:], dbias["b_tau"][:].unsqueeze(0))
                nc.sync.dma_start(bmd[:], dbias["b_mem"][:].unsqueeze(0))
                nc.vector.tensor_copy(btau_row[:], btd[:])
                nc.vector.tensor_copy(bmem_row[:], bmd[:])
                nc.sync.dma_start(
                    b_out_r[:], dbias["b_out"][:].unsqueeze(0)
                    .to_broadcast([128, C]))
                # cdec = b_in @ W_pass + b_pass  (row vector)
                b_in_r = setup_sb.tile([128, H], FP32, name="b_in_r")
                nc.sync.dma_start(
                    b_in_r[:], dbias["b_in"][:].unsqueeze(0)
                    .to_broadcast([128, H]))
                binT = setup_sb.tile([128, KC, 1], FP32R, name="binT")
                binT_d = setup_sb.tile([128, KC, 1], FP32, name="binT_d")
                nc.sync.dma_start(
                    binT_d[:],
                    dbias["b_in"][:].rearrange("(c p) -> p c", p=128)
                    .unsqueeze(2))
                nc.vector.tensor_copy(binT[:], binT_d[:])
                psd = setup_ps.tile([1, H], FP32, tag="setup_D")
                for c in range(KC):
                    nc.tensor.matmul(psd[:], binT[:, c, :], wpass_r[:, c, :],
                                     start=(c == 0), stop=(c == KC - 1))
                bps = setup_sb.tile([1, H], FP32, name="bps")
                nc.sync.dma_start(bps[:], dbias["b_pass"][:].unsqueeze(0))
                nc.vector.tensor_tensor(out=cdec_row[:], in0=psd[:],
                                        in1=bps[:], op=Op.add)

            # x -> xT ; cx/cp/cmix precompute
            x_sb = setup_sb.tile([128, NMT, I], FP32, name="x_sb")
            xT = setup_sb.tile([128, NMT, 128], FP32R, name="xT")
            nc.sync.dma_start(
                x_sb[:],
                dx[:].rearrange("b t i -> (b t) i").rearrange(
                    "(m p) i -> p m i", p=128))
            for m in range(NMT):
                pst = setup_ps.tile([128, 128], FP32, tag="setup_T")
                nc.tensor.transpose(pst[:], x_sb[:, m, :], id128[:])
                nc.vector.tensor_copy(xT[:, m, :], pst[:])
            if not zero_bias:
                b_in_bc = b_in_r
                cdec_bc = setup_sb.tile([128, H], FP32, name="cdec_bc")
                nc.sync.dma_start(
                    cdec_bc[:], dbias["b_pass"][:].unsqueeze(0)
                    .to_broadcast([128, H]))
                # cdec broadcast = b_in@W_pass + b_pass on every partition:
                # recompute via per-partition copy from cdec_row is awkward;
                # use psd result broadcast through DRAM scratch.
                dcdec = nc.dram_tensor("cdec_scratch", [H], FP32)
                nc.sync.dma_start(dcdec[:], f32(cdec_row[:]).squeeze(0))
                cdec_full = setup_sb.tile([128, H], FP32, name="cdec_full")
                nc.sync.dma_start(
                    cdec_full[:], dcdec[:].unsqueeze(0).to_broadcast([128, H]))
            for m in range(NMT):
                ps1 = setup_ps.tile([128, H], FP32, tag="setup_G")
                nc.tensor.matmul(ps1[:], xT[:, m, :], wx_r[:],
                                 start=True, stop=True)
                cxt = setup_sb.tile([128, H], FP32R, tag="cxt", bufs=3)
                if zero_bias:
                    nc.vector.tensor_copy(cxt[:], ps1[:])
                else:
                    nc.vector.tensor_tensor(out=cxt[:], in0=ps1[:],
                                            in1=b_in_bc[:], op=Op.add)
                ps2 = setup_ps.tile([128, H], FP32, tag="setup_G")
                nc.tensor.matmul(ps2[:], xT[:, m, :], wxp[:],
                                 start=True, stop=True)
                cpt = setup_sb.tile([128, H], FP32R, tag="cpt", bufs=3)
                if zero_bias:
                    nc.vector.tensor_copy(cpt[:], ps2[:])
                else:
                    nc.vector.tensor_tensor(out=cpt[:], in0=ps2[:],
                                            in1=cdec_full[:], op=Op.add)
                nc.vector.copy_predicated(
                    out=f32(cxt[:]),
                    mask=mk_bt_i[:, m // TPB, m % TPB].unsqueeze(1)
                    .to_broadcast([128, H]),
                    data=f32(cpt[:]))
                cxb = setup_sb.tile([128, H], BF16, tag="cxb", bufs=3)
                nc.vector.tensor_copy(cxb[:], f32(cxt[:]))
                nc.sync.dma_start(dcmix[bass.ts(m, 128), :], cxb[:])

        # ------------- main recurrence -------------
        psum = ctx.enter_context(tc.tile_pool(name="mn_ps", bufs=1,
                                              space="PSUM"))
        psum2 = ctx.enter_context(tc.tile_pool(name="mn_ps2", bufs=2,
                                               space="PSUM"))
        loop_sb = ctx.enter_context(tc.tile_pool(name="mn_sb", bufs=2))
        dma_sb = ctx.enter_context(tc.tile_pool(name="mn_dma", bufs=4))

        cmix_v = dcmix[:].rearrange("(b t) n -> b t n", b=BL)
        bufs = [buf0, buf1]

        def _main_pass():
            h0 = loop_sb.tile([128, CB], BF16, tag="h0")
            nc.vector.memset(h0[:], 0.0)
            nc.gpsimd.memset(buf0[:], 0.0)

            deferred = []

            buf_idx = 0
            total_steps = seq_len + t_out
            for t in range(total_steps):
                is_enc = t < seq_len
                td = t - seq_len
                last = (t == total_steps - 1)

                if not is_enc:
                    nc.vector.tensor_copy(
                        h0coll[:, :, td, :],
                        h0[:].rearrange("p (c b) -> p c b", c=KC))
                    if last:
                        break

                # stationary for the masked dW stream
                if is_enc:
                    mT = maskR[:, t, :].unsqueeze(1).to_broadcast(
                        [128, KC, BL])
                    bst = loop_sb.tile([128, CB], BF16, tag="bst")
                    nc.vector.tensor_tensor(
                        out=bst[:].rearrange("p (c b) -> p c b", c=KC),
                        in0=h0[:].rearrange("p (c b) -> p c b", c=KC),
                        in1=mT, op=Op.mult)

                # deferred buffer update from the previous step (Pool-heavy)
                while deferred:
                    deferred.pop(0)()

                # ---- L1: single accumulation, cmix first ----
                ps1 = psum.tile([BL, H], FP32, tag="ps1")
                h0_v = h0[:].rearrange("p (c b) -> p c b", c=KC)
                if is_enc:
                    cmix4 = dma_sb.tile([BL, H], BF16, tag="cmix4")
                    nc.sync.dma_start(cmix4[:], cmix_v[:, t, :])
                    bst_v = bst[:].rearrange("p (c b) -> p c b", c=KC)
                    nc.tensor.matmul(ps1[:], i4b[:], cmix4[:],
                                     start=True, stop=False)
                    for c in range(KC):
                        nc.tensor.matmul(ps1[:], h0_v[:, c, :], wh[:, c, :],
                                         start=False, stop=False)
                    for c in range(KC):
                        nc.tensor.matmul(ps1[:], bst_v[:, c, :], wdl[:, c, :],
                                         start=False, stop=(c == KC - 1))
                else:
                    if not zero_bias:
                        nc.tensor.matmul(ps1[:], ones1b[:], cdec_row[:],
                                         start=True, stop=False)
                    for c in range(KC):
                        nc.tensor.matmul(ps1[:], h0_v[:, c, :], wp2[:, c, :],
                                         start=(c == 0 and zero_bias),
                                         stop=(c == KC - 1))

                # h' PSUM -> SBUF: halves on two engines, separate tiles
                h_sb0 = loop_sb.tile([BL, NH], BF16, tag="h_sb0")
                h_sb1 = loop_sb.tile([BL, NH], BF16, tag="h_sb1")
                nc.vector.tensor_copy(h_sb0[:], ps1[:, 0:NH])
                nc.scalar.copy(h_sb1[:], ps1[:, NH:H])

                # T1: h' -> h'^T
                ps_t1 = psum2.tile([128, KC, BL], FP32, tag="ps_T1")
                for c, hsb in ((0, h_sb0), (1, h_sb0), (2, h_sb1),
                               (3, h_sb1)):
                    nc.tensor.matmul(ps_t1[:, c, :],
                                     hsb[:, bass.ts(c % 2, 128)],
                                     i4b[:], start=True, stop=True)
                hT = loop_sb.tile([128, CB], BF16, tag="hT")
                hT_c = hT[:].rearrange("p (c b) -> p c b", c=KC)
                nc.vector.tensor_copy(hT_c[:, 0:2, :], ps_t1[:, 0:2, :])
                nc.vector.tensor_copy(hT_c[:, 2:4, :], ps_t1[:, 2:4, :])

                # ---- L2: tau then mem, separate PSUM banks; T2-tau
                # rides inside the mem stream ----
                ps2a = psum.tile([BL, H], FP32, tag="ps2a")
                ps2b = psum.tile([BL, H], FP32, tag="ps2b")
                hT_v = hT[:].rearrange("p (c b) -> p c b", c=KC)
                for c in range(KC):
                    nc.tensor.matmul(ps2a[:], hT_v[:, c, :], wtau[:, c, :],
                                     start=(c == 0),
                                     stop=(c == KC - 1 and zero_bias))
                if not zero_bias:
                    nc.tensor.matmul(ps2a[:], ones1b[:], btau_row[:],
                                     start=False, stop=True)
                for c in range(3):
                    nc.tensor.matmul(ps2b[:], hT_v[:, c, :], wmem[:, c, :],
                                     start=(c == 0), stop=False)

                # tau copies (halves on two engines) while mem streams
                tau_r0 = loop_sb.tile([BL, NH], BF16, tag="tau_r0")
                tau_r1 = loop_sb.tile([BL, NH], BF16, tag="tau_r1")
                nc.scalar.copy(tau_r0[:], ps2a[:, 0:NH])
                nc.vector.tensor_copy(tau_r1[:], ps2a[:, NH:H])
                ps_t2 = psum.tile([128, 2, KC, BL], FP32, tag="ps_T2")
                for c, tr in ((0, tau_r0), (1, tau_r0), (2, tau_r1),
                              (3, tau_r1)):
                    nc.tensor.matmul(ps_t2[:, 0, c, :],
                                     tr[:, bass.ts(c % 2, 128)], i4b[:],
                                     start=True, stop=True)
                nc.tensor.matmul(ps2b[:], hT_v[:, 3, :], wmem[:, 3, :],
                                 start=False, stop=zero_bias)
                if not zero_bias:
                    nc.tensor.matmul(ps2b[:], ones1b[:], bmem_row[:],
                                     start=False, stop=True)

                sig = loop_sb.tile([128, 2, CB], FP32, tag="sig")
                nc.scalar.activation(sig[:, 0], ps_t2[:, 0].rearrange(
                    "p c b -> p (c b)"), Sig)
                taup = loop_sb.tile([128, CB], FP32, tag="taup")
                nc.vector.tensor_scalar(out=taup[:], in0=sig[:, 0],
                                        scalar1=16.0, scalar2=1.0,
                                        op0=Op.mult, op1=Op.max)
                rtau = loop_sb.tile([128, CB], FP32, tag="rtau")
                nc.vector.reciprocal(out=rtau[:], in_=taup[:])

                # mem copies + T2 + sigmoid + q
                mem_r0 = loop_sb.tile([BL, NH], BF16, tag="mem_r0")
                mem_r1 = loop_sb.tile([BL, NH], BF16, tag="mem_r1")
                nc.scalar.copy(mem_r0[:], ps2b[:, 0:NH])
                nc.vector.tensor_copy(mem_r1[:], ps2b[:, NH:H])
                for c, mr in ((0, mem_r0), (1, mem_r0), (2, mem_r1),
                              (3, mem_r1)):
                    nc.tensor.matmul(ps_t2[:, 1, c, :],
                                     mr[:, bass.ts(c % 2, 128)], i4b[:],
                                     start=True, stop=True)
                nc.scalar.activation(sig[:, 1], ps_t2[:, 1].rearrange(
                    "p c b -> p (c b)"), Sig)
                q = loop_sb.tile([128, CB], FP32, tag="q")
                nc.vector.tensor_tensor(
                    out=q[:], in0=sig[:, 1],
                    in1=ps_t1[:].rearrange("p c b -> p (c b)"), op=Op.mult)
                t1 = loop_sb.tile([128, CB], FP32, tag="t1")
                nc.vector.tensor_tensor(out=t1[:], in0=q[:], in1=rtau[:],
                                        op=Op.mult)
                bcur, bnxt = bufs[buf_idx], bufs[buf_idx ^ 1]
                buf_idx ^= 1
                h0 = loop_sb.tile([128, CB], BF16, tag="h0")
                nc.vector.tensor_tensor(out=h0[:], in0=bcur[:, :, 0],
                                        in1=t1[:], op=Op.add)

                # deferred: buffer weights + shift-add (next step, on Pool)
                if t < total_steps - 2:
                    def make_deferred(taup=taup, q=q, bcur=bcur, bnxt=bnxt):
                        def emit():
                            # 1+|tau-d| = max(tau-(d-1), (d+1)-tau)
                            e0 = loop_sb.tile([128, CB, D], FP32, tag="e0")
                            nc.gpsimd.tensor_tensor(
                                out=e0[:],
                                in0=taup[:].unsqueeze(2).to_broadcast(
                                    [128, CB, D]),
                                in1=iota0[:].unsqueeze(1).to_broadcast(
                                    [128, CB, D]),
                                op=Op.subtract)
                            e1 = loop_sb.tile([128, CB, D], FP32, tag="e1")
                            nc.gpsimd.tensor_tensor(
                                out=e1[:],
                                in0=iota2[:].unsqueeze(1).to_broadcast(
                                    [128, CB, D]),
                                in1=taup[:].unsqueeze(2).to_broadcast(
                                    [128, CB, D]),
                                op=Op.subtract)
                            wa = loop_sb.tile([128, CB, D], FP32, tag="wa")
                            nc.vector.tensor_tensor(
                                out=wa[:], in0=e0[:], in1=e1[:], op=Op.max)
                            wr = loop_sb.tile([128, CB, D], FP32, tag="wr")
                            nc.vector.reciprocal_approx_fast(
                                out=wr[:], in_=wa[:])
                            wq = loop_sb.tile([128, CB, D], FP32, tag="wq")
                            nc.gpsimd.tensor_tensor(
                                out=wq[:], in0=wr[:],
                                in1=q[:].unsqueeze(2).to_broadcast(
                                    [128, CB, D]),
                                op=Op.mult)
                            nc.gpsimd.tensor_tensor(
                                out=bnxt[:, :, 0:D - 1], in0=bcur[:, :, 1:D],
                                in1=wq[:, :, 1:D], op=Op.add)
                            nc.gpsimd.memset(bnxt[:, :, D - 1], 0.0)
                        return emit
                    deferred.append(make_deferred())

            # batched output GEMM: out = h0coll @ W_out + b_out
            rows = t_out * BL
            for start in range(0, rows, 128):
                mrows = min(128, rows - start)
                t0, tn = start // BL, mrows // BL
                ps_o = psum.tile([mrows, C], FP32, tag="ps_o")
                for c in range(KC):
                    nc.tensor.matmul(
                        ps_o[:], h0coll[:, c, t0:t0 + tn, :], wout[:, c, :],
                        start=(c == 0), stop=(c == KC - 1))
                out_sb = loop_sb.tile([mrows, C], FP32, tag="out_sb")
                if zero_bias:
                    nc.vector.tensor_copy(out_sb[:], ps_o[:])
                else:
                    nc.vector.tensor_tensor(out=out_sb[:], in0=ps_o[:],
                                            in1=b_out_r[0:mrows, :],
                                            op=Op.add)
                for tt in range(tn):
                    nc.sync.dma_start(dout[:, t0 + tt, :],
                                      out_sb[bass.ts(tt, BL), :])

        for _rep in range(reps):
            _main_pass()

    nc.compile()
    return nc


_CACHE = {}


def _get_module(seq_len, t_out, zero_bias):
    key = (seq_len, t_out, zero_bias)
    if key not in _CACHE:
        _CACHE[key] = build(seq_len, t_out, zero_bias)
    return _CACHE[key]


def kernel(**inputs):
    x = np.ascontiguousarray(np.asarray(inputs["x"], dtype=np.float32))
    lengths = np.ascontiguousarray(
        np.asarray(inputs["lengths"]).astype(np.int32))
    t_out = int(inputs["out_lengths"])
    seq_len = x.shape[1]
    names = ["W_in", "W_pass", "W_tau", "W_mem", "W_out",
             "b_in", "b_pass", "b_tau", "b_mem", "b_out"]
    warrs = {n: np.ascontiguousarray(np.asarray(inputs[n], dtype=np.float32))
             for n in names}
    zero_bias = all(not np.any(warrs[n]) for n in
                    ["b_in", "b_pass", "b_tau", "b_mem", "b_out"])
    nc = _get_module(seq_len, t_out, zero_bias)

    from concourse import bass_utils
    in_maps = []
    for c in range(NCORES):
        sl = slice(c * BL, (c + 1) * BL)
        m = {"x": x[sl], "lengths": lengths[sl]}
        m.update(warrs)
        in_maps.append(m)
    res = bass_utils.run_bass_kernel_spmd(
        nc, in_maps, core_ids=list(range(NCORES)))
    out = np.concatenate([res.results[c]["out"] for c in range(NCORES)],
                         axis=0)
    return out
